# revision 1
# baseline (speedup 1.0000x reference)
"""CrossPhaseRoutingLayer Trainium2 kernel.

Full inputs -> full output. Data-parallel over the fused B*C=512 sequence axis
across 8 NeuronCores (64 sequences each). Per core, sequences are processed in
groups of G=4 (T = G*96 = 384 token columns per group).

Algebraic restructuring (host-side, weight-only folds, validated to ~6e-6):
  - Sender attention q = router @ Wq_s + bq_s is input-independent, so sender
    scores fold into one matrix: scores^T = M_score^T @ x^T + c_score, where
    M_score[d,(h,r)] = Wk_s[d,h-slice] . q_s[r,h-slice] / sqrt(E).
  - The sender value/output path runs in "mix first, project later" order:
    T_mix = A1 @ x (per head/router), then per-head Wv_s slice, then Wo_s.
    Sender biases collapse: c_send = bv_s @ Wo_s + bo_s.
  - Receiver: scale folds into Wq_r/bq_r; bv_r folds into c_recv = bv_r@Wo_r+bo_r.
  - Softmaxes skip max-subtraction (scores provably tiny: |s| < 0.1).

On-chip layout: activations live transposed (x^T: [D-chunk(128 part), token])
for all D-contraction matmuls; token-partition tiles where per-token free-dim
reductions (softmax) or token-contraction (A1 @ x) are needed; PE transposes
convert. Big matmuls (N>=256) run as float32r (~2.7x fp32 rate, rel err ~1e-4).
LayerNorm over the partitioned D axis uses ones-matmul reductions and a
[K=1] broadcast matmul.
"""
import numpy as np

import concourse.bacc as bacc
import concourse.bass as bass
import concourse.mybir as mybir
import concourse.tile as tile
from concourse.bass_utils import run_bass_kernel_spmd
from concourse.masks import make_identity

FP = mybir.dt.float32
FPR = mybir.dt.float32r
AX = mybir.AxisListType
OP = mybir.AluOpType
ACTF = mybir.ActivationFunctionType

B, C, L, D = 16, 32, 96, 512
R, H = 8, 4
E = D // H            # 128
HR = H * R            # 32
DC = D // 128         # 4 D-chunks
OC = (4 * D) // 128   # 16 MLP hidden chunks
EPS = 1e-5
N_CORES = 8
G = 4                 # sequences per group
T = G * L             # 384 token columns per group

W_NAMES = ["Msc", "Wv_s", "Wo_s", "Wq_r", "Wk_r", "Wv_r", "Wo_r", "W1", "W2"]
V_NAMES = ["c_score", "c_send", "c_recv", "bq_r", "bk_r", "b1", "b2",
           "ln1_g", "ln1_b", "ln2_g", "ln2_b"]


def build_core_kernel(n_seq: int):
    """Bass program for one core processing n_seq sequences."""
    assert n_seq % G == 0
    n_groups = n_seq // G
    nc = bacc.Bacc(None)

    z = nc.declare_dram_parameter("z", [n_seq * L, D], FPR, isOutput=False)
    out = nc.declare_dram_parameter("out", [n_seq * L, D], FP, isOutput=True)
    wd = {}
    for name, shape in [("Msc", [D, HR]), ("Wv_s", [D, D]), ("Wo_s", [D, D]),
                        ("Wq_r", [D, D]), ("Wk_r", [D, D]), ("Wv_r", [D, D]),
                        ("Wo_r", [D, D]), ("W1", [D, 4 * D]), ("W2", [4 * D, D])]:
        wd[name] = nc.declare_dram_parameter(name, shape, FPR, isOutput=False)
    for name, n in [("c_score", HR), ("c_send", D), ("c_recv", D), ("bq_r", D),
                    ("bk_r", D), ("b1", 4 * D), ("b2", D), ("ln1_g", D),
                    ("ln1_b", D), ("ln2_g", D), ("ln2_b", D)]:
        wd[name] = nc.declare_dram_parameter(name, [n], FP, isOutput=False)

    with tile.TileContext(nc) as tc:
        with tc.tile_pool(name="wpool", bufs=1) as wp, \
             tc.tile_pool(name="xin", bufs=2) as px, \
             tc.tile_pool(name="act1", bufs=1) as pa, \
             tc.tile_pool(name="sm", bufs=2) as psm, \
             tc.tile_pool(name="micro", bufs=3) as pmi, \
             tc.tile_pool(name="big3", bufs=3) as pb3, \
             tc.tile_pool(name="otok", bufs=1) as po, \
             tc.tile_pool(name="ps", bufs=1, space="PSUM") as ps:

            # ---------------- resident weights / constants -----------------
            w = {}
            w["Msc"] = wp.tile([128, DC, HR], FPR, name="w_Msc")
            for name in ["Wv_s", "Wo_s", "Wq_r", "Wk_r", "Wv_r", "Wo_r"]:
                w[name] = wp.tile([128, DC, D], FPR, name=f"w_{name}")
            w["W1"] = wp.tile([128, DC, 4 * D], FPR, name="w_W1")
            w["W2"] = wp.tile([128, OC, D], FPR, name="w_W2")
            for name in W_NAMES:
                nc.sync.dma_start(
                    out=w[name],
                    in_=wd[name].rearrange("(c p) x -> p c x", p=128))
            w["c_score"] = wp.tile([HR, 1], FP, name="w_c_score")
            nc.sync.dma_start(out=w["c_score"],
                              in_=wd["c_score"].rearrange("(p o) -> p o", o=1))
            for name in ["c_send", "c_recv", "bq_r", "bk_r", "b2",
                         "ln1_g", "ln1_b", "ln2_g", "ln2_b"]:
                w[name] = wp.tile([128, DC], FP, name=f"w_{name}")
                nc.sync.dma_start(out=w[name],
                                  in_=wd[name].rearrange("(c p) -> p c", p=128))
            w["b1"] = wp.tile([128, OC], FP, name="w_b1")
            nc.sync.dma_start(out=w["b1"],
                              in_=wd["b1"].rearrange("(c p) -> p c", p=128))

            ident = wp.tile([128, 128], FP, name="ident")
            make_identity(nc, ident)
            identr = wp.tile([128, 128], FPR, name="identr")
            nc.scalar.copy(out=identr, in_=ident)
            ones_f = wp.tile([128, 1], FP, name="ones_f")
            nc.vector.memset(ones_f, 1.0)
            ones_r = wp.tile([128, 1], FPR, name="ones_r")
            nc.scalar.copy(out=ones_r, in_=ones_f)
            eps_t = wp.tile([1, 1], FP, name="eps_t")
            nc.vector.memset(eps_t, EPS)

            for gi in range(n_groups):
                group_body(nc, tc, w, ident, identr, ones_r, eps_t,
                           z, out, gi,
                           px, pa, psm, pmi, pb3, po, ps)
    nc.finalize()
    return nc


def layernorm_T(nc, w, ones_r, eps_t, pmi, pb3, ps,
                s_T, out_tile, g_name, b_name, out_dtype, tag):
    """LN over the partition-split D axis of s_T [128, DC, T] -> out_tile."""
    mean_ps = ps.tile([1, T], FP, name=f"mean_ps{tag}", tag="big", bufs=2)
    for k in range(DC):
        nc.tensor.matmul(out=mean_ps, lhsT=ones_r, rhs=s_T[:, k, :],
                         start=(k == 0), stop=(k == DC - 1))
    msc = pmi.tile([1, T], FP, name=f"msc{tag}", tag="micro")
    nc.scalar.activation(out=msc, in_=mean_ps, func=ACTF.Copy, scale=1.0 / D)

    ss_ps = ps.tile([1, T], FP, name=f"ss_ps{tag}", tag="big", bufs=2)
    for k in range(DC):
        sq = pb3.tile([128, T], FPR, name=f"sq{tag}", tag="sq", bufs=2)
        nc.vector.tensor_mul(out=sq, in0=s_T[:, k, :].bitcast(FP),
                             in1=s_T[:, k, :].bitcast(FP))
        nc.tensor.matmul(out=ss_ps, lhsT=ones_r, rhs=sq,
                         start=(k == 0), stop=(k == DC - 1))

    msc2 = pmi.tile([1, T], FP, name=f"msc2{tag}", tag="micro")
    nc.vector.tensor_mul(out=msc2, in0=msc, in1=msc)
    var_s = pmi.tile([1, T], FP, name=f"var{tag}", tag="micro")
    nc.vector.scalar_tensor_tensor(out=var_s, in0=ss_ps, scalar=1.0 / D,
                                   in1=msc2, op0=OP.mult, op1=OP.subtract)
    srt = pmi.tile([1, T], FP, name=f"srt{tag}", tag="micro")
    nc.scalar.activation(out=srt, in_=var_s, func=ACTF.Sqrt, bias=eps_t)
    rstd = pmi.tile([1, T], FP, name=f"rstd{tag}", tag="micro")
    nc.vector.reciprocal(out=rstd, in_=srt)
    mr = pmi.tile([1, T], FP, name=f"mr{tag}", tag="micro")
    nc.vector.tensor_mul(out=mr, in0=msc, in1=rstd)

    rstdB = pb3.tile([128, T], FP, name=f"rstdB{tag}", tag="rstdB", bufs=2)
    nc.gpsimd.partition_broadcast(rstdB, rstd)
    mrB = pb3.tile([128, T], FP, name=f"mrB{tag}", tag="mrB", bufs=2)
    nc.gpsimd.partition_broadcast(mrB, mr)

    for k in range(DC):
        t1 = pb3.tile([128, T], FP, name=f"t1{tag}", tag="lnt", bufs=2)
        nc.vector.tensor_mul(out=t1, in0=s_T[:, k, :].bitcast(FP), in1=rstdB)
        nc.vector.tensor_sub(out=t1, in0=t1, in1=mrB)
        nc.vector.tensor_scalar(out=out_tile[:, k, :],
                                in0=t1,
                                scalar1=w[g_name][:, k:k + 1], op0=OP.mult,
                                scalar2=w[b_name][:, k:k + 1], op1=OP.add)


def group_body(nc, tc, w, ident, identr, ones_r, eps_t, z, out, gi,
               px, pa, psm, pmi, pb3, po, ps):
    r0 = gi * T   # first DRAM row of the group

    # ---- load x (token-partition) and build x^T ----
    x_tok = px.tile([L, G, D], FPR, name="x_tok")
    nc.sync.dma_start(out=x_tok,
                      in_=z[r0:r0 + T, :].rearrange("(g l) d -> l g d", g=G))
    xT = pa.tile([128, DC, T], FPR, name="xT")
    for g in range(G):
        for dc in range(DC):
            pt = ps.tile([128, L], FPR, name="pt_x", tag="sp", bufs=2)
            nc.tensor.transpose(out=pt, in_=x_tok[:, g, dc * 128:(dc + 1) * 128],
                                identity=identr[:L, :L])
            nc.scalar.copy(out=xT[:, dc, g * L:(g + 1) * L], in_=pt)
    xTr = xT  # FPR view; read with .bitcast(FP)

    # ---- sender scores^T [HR, T] and softmax over tokens ----
    sc_ps = ps.tile([HR, T], FP, name="sc_ps", tag="big", bufs=2)
    for k in range(DC):
        nc.tensor.matmul(out=sc_ps, lhsT=w["Msc"][:, k, :], rhs=xTr[:, k, :],
                         start=(k == 0), stop=(k == DC - 1))
    e1 = psm.tile([HR, T], FP, name="e1")
    nc.scalar.activation(out=e1, in_=sc_ps, func=ACTF.Exp, bias=w["c_score"])
    s1sum = psm.tile([HR, G], FP, name="s1sum")
    nc.vector.tensor_reduce(out=s1sum, in_=e1.rearrange("p (g l) -> p g l", g=G),
                            axis=AX.X, op=OP.add)
    r1 = psm.tile([HR, G], FP, name="r1")
    nc.vector.reciprocal(out=r1, in_=s1sum)

    # A1^T per sequence (token-partition), un/normalized handling:
    # normalize in [HR, L] layout then transpose to [L, HR].
    a1t = []
    for g in range(G):
        a1n = psm.tile([HR, L], FP, name=f"a1n{g}", tag="a1n", bufs=2)
        nc.vector.tensor_scalar_mul(out=a1n, in0=e1[:, g * L:(g + 1) * L],
                                    scalar1=r1[:, g:g + 1])
        a1p = ps.tile([L, HR], FP, name="a1p", tag="sp", bufs=2)
        nc.tensor.transpose(out=a1p, in_=a1n, identity=ident[:HR, :HR])
        a1s = psm.tile([L, HR], FPR, name=f"a1s{g}", tag="a1s", bufs=4)
        nc.scalar.copy(out=a1s, in_=a1p)
        a1t.append(a1s)

    # ---- T_mix^T [(dc), (g, hr)] = x_chunk.T @ A1^T  (contract tokens) ----
    tm_ps = ps.tile([128, DC, G, HR], FP, name="tm_ps", tag="sp", bufs=2)
    for g in range(G):
        for dc in range(DC):
            nc.tensor.matmul(out=tm_ps[:, dc, g, :],
                             lhsT=x_tok[:, g, dc * 128:(dc + 1) * 128],
                             rhs=a1t[g], start=True, stop=True)
    TmT = pa.tile([128, DC, G, HR], FPR, name="TmT")
    nc.scalar.copy(out=TmT, in_=tm_ps)

    # ---- out_cat^T chunk h = Wv_s_h^T @ Tm_h^T   [128,(g,r)] ----
    oc_ps = ps.tile([128, H, G, R], FP, name="oc_ps", tag="sp", bufs=2)
    for h in range(H):
        for k in range(DC):
            nc.tensor.matmul(out=oc_ps[:, h, :, :],
                             lhsT=w["Wv_s"][:, k, h * E:(h + 1) * E],
                             rhs=TmT[:, k, :, h * R:(h + 1) * R],
                             start=(k == 0), stop=(k == DC - 1))
    Oc = pa.tile([128, H, G, R], FPR, name="Oc")
    nc.scalar.copy(out=Oc, in_=oc_ps)

    # ---- router_buffer^T [(dc), (g, r)] = Wo_s^T @ out_cat^T + c_send ----
    rb_ps = ps.tile([128, DC, G, R], FP, name="rb_ps", tag="sp", bufs=2)
    for dc in range(DC):
        for k in range(DC):
            nc.tensor.matmul(out=rb_ps[:, dc, :, :],
                             lhsT=w["Wo_s"][:, k, dc * 128:(dc + 1) * 128],
                             rhs=Oc[:, k, :, :],
                             start=(k == 0), stop=(k == DC - 1))
    rb = pa.tile([128, DC, G, R], FPR, name="rb")
    for dc in range(DC):
        nc.scalar.activation(out=rb[:, dc, :, :],
                             in_=rb_ps[:, dc, :, :], func=ACTF.Identity,
                             bias=w["c_send"][:, dc:dc + 1])

    # ---- receiver k^T [(dc=head), (g,r)] ----
    kt_ps = ps.tile([128, DC, G, R], FP, name="kt_ps", tag="sp", bufs=2)
    for dc in range(DC):
        for k in range(DC):
            nc.tensor.matmul(out=kt_ps[:, dc, :, :],
                             lhsT=w["Wk_r"][:, k, dc * 128:(dc + 1) * 128],
                             rhs=rb[:, k, :, :],
                             start=(k == 0), stop=(k == DC - 1))
    kT = pa.tile([128, DC, G, R], FPR, name="kT")
    for dc in range(DC):
        nc.scalar.activation(out=kT[:, dc, :, :],
                             in_=kt_ps[:, dc, :, :], func=ACTF.Identity,
                             bias=w["bk_r"][:, dc:dc + 1])

    # ---- receiver v in router-partition layout [8, D] per seq (bias folded) ----
    v_sb = []
    for g in range(G):
        v_ps = ps.tile([R, D], FP, name="v_ps", tag="sp", bufs=2)
        for k in range(DC):
            nc.tensor.matmul(out=v_ps, lhsT=rb[:, k, g, :],
                             rhs=w["Wv_r"][:, k, :],
                             start=(k == 0), stop=(k == DC - 1))
        v_g = psm.tile([R, D], FPR, name=f"v_g{g}", tag="v_g", bufs=4)
        nc.scalar.copy(out=v_g, in_=v_ps)
        v_sb.append(v_g)

    # ---- receiver q^T [(dc), T] (scale+bias pre-folded) ----
    qT = pa.tile([128, DC, T], FPR, name="qT", tag="big_a")
    for dc in range(DC):
        q_ps = ps.tile([128, T], FP, name="q_ps", tag="big", bufs=2)
        for k in range(DC):
            nc.tensor.matmul(out=q_ps, lhsT=w["Wq_r"][:, k, dc * 128:(dc + 1) * 128],
                             rhs=xTr[:, k, :], start=(k == 0), stop=(k == DC - 1))
        nc.scalar.activation(out=qT[:, dc, :], in_=q_ps,
                             func=ACTF.Identity, bias=w["bq_r"][:, dc:dc + 1])

    # ---- receiver scores -> softmax -> mix (dual-layout, no transposes) ----
    aT = pa.tile([128, DC, T], FPR, name="aT", tag="big_b")
    for g in range(G):
        # token-partition scores for the softmax denominators
        s2_ps = ps.tile([L, H, R], FP, name="s2_ps", tag="sp", bufs=2)
        for h in range(H):
            nc.tensor.matmul(out=s2_ps[:, h, :],
                             lhsT=qT[:, h, g * L:(g + 1) * L],
                             rhs=kT[:, h, g, :], start=True, stop=True)
        e2 = psm.tile([L, H, R], FP, name=f"e2{g}", tag="e2", bufs=2)
        nc.scalar.activation(out=e2, in_=s2_ps, func=ACTF.Exp)
        ssum = psm.tile([L, H], FP, name=f"ssum{g}", tag="ssum", bufs=2)
        nc.vector.tensor_reduce(out=ssum, in_=e2, axis=AX.X, op=OP.add)
        r2 = psm.tile([L, H], FP, name=f"r2{g}", tag="r2", bufs=2)
        nc.vector.reciprocal(out=r2, in_=ssum)
        for h in range(H):
            # r2 column h -> [1, L] at partition 0, broadcast on gpsimd
            r2p = ps.tile([1, L], FP, name="r2p", tag="sp", bufs=2)
            nc.tensor.transpose(out=r2p, in_=r2[:, h:h + 1], identity=ident[:L, :L])
            r2T = psm.tile([1, L], FP, name=f"r2T{g}{h}", tag="r2T", bufs=2)
            nc.scalar.copy(out=r2T, in_=r2p)
            # router-partition scores -> exp directly (same math, swapped operands)
            s2t_ps = ps.tile([R, L], FP, name="s2t_ps", tag="sp", bufs=2)
            nc.tensor.matmul(out=s2t_ps, lhsT=kT[:, h, g, :],
                             rhs=qT[:, h, g * L:(g + 1) * L], start=True, stop=True)
            e2t = psm.tile([R, L], FPR, name=f"e2t{g}{h}", tag="e2t", bufs=2)
            nc.scalar.activation(out=e2t, in_=s2t_ps, func=ACTF.Exp)
            r2B = pb3.tile([128, L], FP, name=f"r2B{g}{h}", tag="r2B", bufs=2)
            nc.gpsimd.partition_broadcast(r2B, r2T)
            # mix: apT chunk (head h) = v_h^T-as-lhsT @ e2t, normalize on copy-out
            apT_ps = ps.tile([128, L], FP, name="apT_ps", tag="sp", bufs=2)
            nc.tensor.matmul(out=apT_ps, lhsT=v_sb[g][:, h * E:(h + 1) * E],
                             rhs=e2t, start=True, stop=True)
            nc.vector.tensor_mul(out=aT[:, h, g * L:(g + 1) * L],
                                 in0=apT_ps, in1=r2B)

    # ---- attn2^T = Wo_r^T @ attn_pre^T + c_recv; residual; LN1 ----
    s1T = pa.tile([128, DC, T], FPR, name="s1T", tag="big_a")
    for dc in range(DC):
        at2_ps = ps.tile([128, T], FP, name="at2_ps", tag="big", bufs=2)
        for k in range(DC):
            nc.tensor.matmul(out=at2_ps,
                             lhsT=w["Wo_r"][:, k, dc * 128:(dc + 1) * 128],
                             rhs=aT[:, k, :], start=(k == 0), stop=(k == DC - 1))
        nc.vector.scalar_tensor_tensor(out=s1T[:, dc, :],
                                       in0=at2_ps,
                                       scalar=w["c_recv"][:, dc:dc + 1],
                                       in1=xTr[:, dc, :].bitcast(FP),
                                       op0=OP.add, op1=OP.add)
    out1T = pa.tile([128, DC, T], FPR, name="out1T", tag="big_b")
    layernorm_T(nc, w, ones_r, eps_t, pmi, pb3, ps,
                s1T, out1T, "ln1_g", "ln1_b", FPR, f"_l1_{gi}")

    # ---- MLP ----
    h2_ps = [ps.tile([128, T], FP, name=f"h2_ps{dc}", tag=f"h2_{dc}", bufs=1)
             for dc in range(DC)]
    for oc in range(OC):
        h1_ps = ps.tile([128, T], FP, name="h1_ps", tag="big", bufs=2)
        for k in range(DC):
            nc.tensor.matmul(out=h1_ps,
                             lhsT=w["W1"][:, k, oc * 128:(oc + 1) * 128],
                             rhs=out1T[:, k, :], start=(k == 0), stop=(k == DC - 1))
        gl = pb3.tile([128, T], FPR, name="gl", tag="gl")
        nc.scalar.activation(out=gl, in_=h1_ps, func=ACTF.Gelu,
                             bias=w["b1"][:, oc:oc + 1])
        for dc in range(DC):
            nc.tensor.matmul(out=h2_ps[dc],
                             lhsT=w["W2"][:, oc, dc * 128:(dc + 1) * 128],
                             rhs=gl, start=(oc == 0), stop=(oc == OC - 1))

    # ---- residual2 + LN2 -> outT (fp32, for output transposes) ----
    s2T = pa.tile([128, DC, T], FPR, name="s2T", tag="big_a")
    for dc in range(DC):
        nc.vector.scalar_tensor_tensor(out=s2T[:, dc, :],
                                       in0=h2_ps[dc],
                                       scalar=w["b2"][:, dc:dc + 1],
                                       in1=out1T[:, dc, :].bitcast(FP),
                                       op0=OP.add, op1=OP.add)
    outT = pa.tile([128, DC, T], FPR, name="outT", tag="outT")
    layernorm_T(nc, w, ones_r, eps_t, pmi, pb3, ps,
                s2T, outT, "ln2_g", "ln2_b", FP, f"_l2_{gi}")

    # ---- transpose back to token rows and store ----
    out_tok = po.tile([128, T // 128, D], FP, name="out_tok")
    for a in range(T // 128):
        for dc in range(DC):
            op_ps = ps.tile([128, 128], FPR, name="op_ps", tag="sp", bufs=2)
            nc.tensor.transpose(out=op_ps,
                                in_=outT[:, dc, a * 128:(a + 1) * 128],
                                identity=identr)
            nc.scalar.copy(out=out_tok[:, a, dc * 128:(dc + 1) * 128],
                           in_=op_ps.bitcast(FP))
    nc.gpsimd.dma_start(out=out[r0:r0 + T, :].rearrange("(a p) d -> p a d", p=128),
                        in_=out_tok)


def _host_fold(inputs):
    """Host-side weight-only precomputation."""
    f32 = np.float32
    scale = 1.0 / np.sqrt(np.float32(E))
    q_s = (inputs["router"] @ inputs["Wq_s"] + inputs["bq_s"]).astype(f32)
    q_sh = q_s.reshape(R, H, E)
    Wk = inputs["Wk_s"].reshape(D, H, E)
    M_score = (np.einsum("dhe,rhe->dhr", Wk, q_sh).reshape(D, HR) * scale).astype(f32)
    c_score = (np.einsum("he,rhe->hr", inputs["bk_s"].reshape(H, E), q_sh)
               .reshape(HR) * scale).astype(f32)
    c_send = (inputs["bv_s"] @ inputs["Wo_s"] + inputs["bo_s"]).astype(f32)
    c_recv = (inputs["bv_r"] @ inputs["Wo_r"] + inputs["bo_r"]).astype(f32)
    return {
        "Msc": np.ascontiguousarray(M_score),
        "c_score": c_score,
        "c_send": c_send,
        "c_recv": c_recv,
        "Wv_s": np.ascontiguousarray(inputs["Wv_s"].astype(f32)),
        "Wo_s": np.ascontiguousarray(inputs["Wo_s"].astype(f32)),
        "Wq_r": np.ascontiguousarray((inputs["Wq_r"] * scale).astype(f32)),
        "bq_r": (inputs["bq_r"] * scale).astype(f32),
        "Wk_r": np.ascontiguousarray(inputs["Wk_r"].astype(f32)),
        "bk_r": inputs["bk_r"].astype(f32),
        "Wv_r": np.ascontiguousarray(inputs["Wv_r"].astype(f32)),
        "Wo_r": np.ascontiguousarray(inputs["Wo_r"].astype(f32)),
        "W1": np.ascontiguousarray(inputs["W1"].astype(f32)),
        "b1": inputs["b1"].astype(f32),
        "W2": np.ascontiguousarray(inputs["W2"].astype(f32)),
        "b2": inputs["b2"].astype(f32),
        "ln1_g": inputs["ln1_g"].astype(f32),
        "ln1_b": inputs["ln1_b"].astype(f32),
        "ln2_g": inputs["ln2_g"].astype(f32),
        "ln2_b": inputs["ln2_b"].astype(f32),
    }


def kernel(**inputs) -> np.ndarray:
    inputs = {k: np.asarray(v) for k, v in inputs.items()}
    Z = inputs["Z"].astype(np.float32)
    n_seq_total = B * C
    n_seq = n_seq_total // N_CORES
    folded = _host_fold(inputs)

    nc = build_core_kernel(n_seq)
    Zf = Z.reshape(n_seq_total, L, D)
    in_maps = []
    for c in range(N_CORES):
        m = {"z": np.ascontiguousarray(
            Zf[c * n_seq:(c + 1) * n_seq].reshape(n_seq * L, D))}
        m.update(folded)
        in_maps.append(m)
    res = run_bass_kernel_spmd(nc, in_maps, list(range(N_CORES)))
    out = np.empty((n_seq_total, L, D), np.float32)
    for c in range(N_CORES):
        out[c * n_seq:(c + 1) * n_seq] = res.results[c]["out"].reshape(n_seq, L, D)
    return out.reshape(B, C, L, D)


if __name__ == "__main__":
    import reference
    inputs = reference.setup_inputs()
    inputs = {k: np.asarray(v) for k, v in inputs.items()}
    expected = np.asarray(reference.reference(**inputs))
    got = kernel(**inputs)
    err = np.abs(got - expected).max()
    rel = err / np.abs(expected).max()
    print(f"abs err {err:.3e}  absmax-rel {rel:.3e}")



# revision 56
# speedup vs baseline: 1.9651x; 1.9651x over previous
"""CrossPhaseRoutingLayer Trainium2 kernel (v3: bf16 + software pipelining).

Full inputs -> full output. Data-parallel over the fused B*C=512 sequence axis
across 8 NeuronCores (64 sequences each); per core, 16 groups of G=4 sequences
(T = 384 token columns per group).

Design highlights:
  - all matmul operands bf16 (4x faster small matmuls, FWL weight loads);
    residual/LN tensors fp32 (fp32r for matmul reads)
  - host-side pre-transpose of Z in both layouts (no on-chip input/output
    transposes); output stored transposed fp16, untransposed on host
  - sender scores collapse into one matrix (router q is input-independent);
    sender value/output path runs mix-first; receiver attention batched over
    the 4 sequences with a block-diagonal exp mask (12 matmuls/group)
  - softmax/LN reciprocals via DVE reciprocal_approx_fast (no ACT table
    thrash; DVE reciprocal on [1,T] rows is ~3.2us each)
  - LN1 gamma/beta folded into W1/b1/b2; LN2 affine applied on host
  - group phases software-pipelined at emission time so group i's MLP
    matmuls hide group i+1's softmax/LN cross-engine latency (the PE
    executes its stream in order - without interleaving every ACT/DVE
    round-trip stalls it)
"""
import numpy as np

import concourse.bacc as bacc
import concourse.bass as bass
import concourse.mybir as mybir
import concourse.tile as tile
from concourse.bass_utils import run_bass_kernel_spmd
from concourse.masks import make_identity

FP = mybir.dt.float32
FPR = mybir.dt.float32r
BF = mybir.dt.bfloat16
F16 = mybir.dt.float16
AX = mybir.AxisListType
OP = mybir.AluOpType
ACTF = mybir.ActivationFunctionType

B, C, L, D = 16, 32, 96, 512
R, H = 8, 4
E = D // H            # 128
HR = H * R            # 32
GR = 32               # G * R
DC = D // 128         # 4 chunks of D
OC = (4 * D) // 128   # 16 MLP hidden chunks
EPS = 1e-5
N_CORES = 8
G = 4                 # sequences per group
T = G * L             # 384 token columns per group

BF_NP = mybir.dt.np(BF)
F16_NP = mybir.dt.np(F16)

W_BF = [("Msc", [D, HR]), ("Wv_s", [D, D]), ("Wo_s", [D, D]),
        ("Wq_r", [D, D]), ("Wk_r", [D, D]), ("Wv_r", [D, D]),
        ("Wo_r", [D, D]), ("W1", [D, 4 * D]), ("W2", [4 * D, D])]
V_FP = [("c_score", HR), ("c_send", D), ("c_recv", D), ("bq_r", D),
        ("bk_r", D), ("b1", 4 * D), ("ln1_g", D), ("b2p", D)]


class Cx:
    """Shared emission context (engines, weights, pools, dram handles)."""
    pass


def build_core_kernel(n_seq: int):
    assert n_seq % G == 0
    n_groups = n_seq // G
    nc = bacc.Bacc(None)

    cx = Cx()
    cx.nc = nc
    cx.zt = nc.declare_dram_parameter("zt", [n_groups * 128, DC * T], BF,
                                      isOutput=False)
    cx.ztf = nc.declare_dram_parameter("ztf", [n_groups * 128, DC * T], FP,
                                       isOutput=False)
    cx.ztok = nc.declare_dram_parameter("ztok", [n_groups * L, G * D], BF,
                                        isOutput=False)
    cx.out = nc.declare_dram_parameter("out", [n_groups * 128, DC * T], F16,
                                       isOutput=True)
    wd = {}
    for name, shape in W_BF:
        wd[name] = nc.declare_dram_parameter(name, shape, BF, isOutput=False)
    for name, n in V_FP:
        wd[name] = nc.declare_dram_parameter(name, [n], FP, isOutput=False)
    wd["mask"] = nc.declare_dram_parameter("mask", [HR, T], BF, isOutput=False)

    with tile.TileContext(nc) as tc:
        with tc.tile_pool(name="wpool", bufs=1) as wp, \
             tc.tile_pool(name="xin", bufs=2) as px, \
             tc.tile_pool(name="act", bufs=2) as pa, \
             tc.tile_pool(name="sm", bufs=2) as psm, \
             tc.tile_pool(name="micro", bufs=2) as pmi, \
             tc.tile_pool(name="brd", bufs=2) as pb, \
             tc.tile_pool(name="otok", bufs=2) as po, \
             tc.tile_pool(name="ps", bufs=1, space="PSUM") as ps:

            cx.px, cx.pa, cx.psm, cx.pmi, cx.pb, cx.po, cx.ps = \
                px, pa, psm, pmi, pb, po, ps

            # ------------- resident weights / constants (scalar queue) ------
            w = {}
            w["Msc"] = wp.tile([128, DC, HR], BF, name="w_Msc")
            for name in ["Wv_s", "Wo_s", "Wq_r", "Wk_r", "Wv_r", "Wo_r"]:
                w[name] = wp.tile([128, DC, D], BF, name=f"w_{name}")
            w["W1"] = wp.tile([128, DC, 4 * D], BF, name="w_W1")
            w["W2"] = wp.tile([128, OC, D], BF, name="w_W2")
            for name, _ in W_BF:
                nc.scalar.dma_start(
                    out=w[name],
                    in_=wd[name].rearrange("(c p) x -> p c x", p=128))
            w["c_score"] = wp.tile([HR, 1], FP, name="w_c_score")
            nc.scalar.dma_start(out=w["c_score"],
                                in_=wd["c_score"].rearrange("(p o) -> p o", o=1))
            for name in ["c_send", "c_recv", "bq_r", "bk_r", "ln1_g", "b2p"]:
                w[name] = wp.tile([128, DC], FP, name=f"w_{name}")
                nc.scalar.dma_start(out=w[name],
                                    in_=wd[name].rearrange("(c p) -> p c", p=128))
            w["b1"] = wp.tile([128, OC], FP, name="w_b1")
            nc.scalar.dma_start(out=w["b1"],
                                in_=wd["b1"].rearrange("(c p) -> p c", p=128))
            cx.w = w

            ident = wp.tile([128, 128], FP, name="ident")
            make_identity(nc, ident)
            cx.identb = wp.tile([128, 128], BF, name="identb")
            nc.scalar.copy(out=cx.identb, in_=ident)
            cx.ones32 = wp.tile([HR, 1], BF, name="ones32")
            nc.vector.memset(cx.ones32, 1.0)
            cx.invD = wp.tile([128, 1], BF, name="invD")
            nc.vector.memset(cx.invD, 1.0 / D)
            invDf = wp.tile([128, 1], FP, name="invDf")
            nc.vector.memset(invDf, 1.0 / D)
            cx.invDr = invDf.bitcast(FPR)
            cx.mask32 = wp.tile([HR, T], BF, name="mask32")
            nc.scalar.dma_start(out=cx.mask32, in_=wd["mask"][:, :])

            # ---------------- software-pipelined group loop -----------------
            sts = {0: phase_a(cx, 0)}
            for gi in range(n_groups):
                st = sts[gi]
                phase_b_scores(cx, st)
                if gi > 0:
                    phase_c_mlp(cx, sts[gi - 1], 0, OC // 2)
                phase_b_mix(cx, st)
                if gi > 0:
                    phase_c_tail(cx, sts[gi - 1])
                if gi + 1 < n_groups:
                    sts[gi + 1] = phase_a(cx, gi + 1)
                phase_b_tail(cx, st)
                if gi - 1 in sts:
                    del sts[gi - 1]
            phase_c_mlp(cx, sts[n_groups - 1], 0, OC // 2)
            phase_c_tail(cx, sts[n_groups - 1])
    nc.finalize()
    return nc


def ln_norm(cx, src, dst, tag, dst2=None):
    """dst = (src - mean)/sqrt(var+eps) over the partition-split D axis.

    src: [128, DC, T] fp32r; dst: [128, DC, T]. If dst2 is given, the result
    is also ACT-copied there (bf16 matmul view next to the fp32 dst).
    """
    nc, pmi, pb, ps = cx.nc, cx.pmi, cx.pb, cx.ps
    m_ps = ps.tile([1, T], FP, name=f"m_ps{tag}", tag="sp", bufs=2)
    for k in range(DC):
        nc.tensor.matmul(out=m_ps, lhsT=cx.invDr, rhs=src[:, k, :],
                         start=(k == 0), stop=(k == DC - 1))
    s_ps = ps.tile([1, T], FP, name=f"s_ps{tag}", tag="sp", bufs=2)
    for k in range(DC):
        sq = pb.tile([128, T], BF, name=f"sq{tag}", tag="sq", bufs=2)
        nc.vector.tensor_mul(out=sq, in0=src[:, k, :].bitcast(FP),
                             in1=src[:, k, :].bitcast(FP))
        nc.tensor.matmul(out=s_ps, lhsT=cx.invD, rhs=sq,
                         start=(k == 0), stop=(k == DC - 1))
    m2 = pmi.tile([1, T], FP, name=f"m2{tag}", tag="microln")
    nc.scalar.square(out=m2, in_=m_ps)
    veps = pmi.tile([1, T], FP, name=f"veps{tag}", tag="microln")
    nc.vector.scalar_tensor_tensor(out=veps, in0=s_ps, scalar=EPS,
                                   in1=m2, op0=OP.add, op1=OP.subtract)
    srow = pmi.tile([1, T], FP, name=f"srow{tag}", tag="microln")
    nc.scalar.sqrt(out=srow, in_=veps)
    lrow = pmi.tile([1, 2, T], FP, name=f"lrow{tag}", tag="lrow")
    nc.vector.reciprocal_approx_fast(out=lrow[:, 1, :], in_=srow)
    nc.vector.tensor_mul(out=lrow[:, 0, :], in0=m_ps, in1=lrow[:, 1, :])
    lnB = pb.tile([128, 2, T], FP, name=f"lnB{tag}", tag="lnB", bufs=2)
    nc.gpsimd.partition_broadcast(lnB, lrow)
    for k in range(DC):
        t = pb.tile([128, T], FP, name=f"lt{tag}", tag="lnt", bufs=2)
        nc.vector.tensor_mul(out=t, in0=src[:, k, :].bitcast(FP),
                             in1=lnB[:, 1, :])
        nc.vector.tensor_sub(out=dst[:, k, :], in0=t, in1=lnB[:, 0, :])
        if dst2 is not None:
            nc.scalar.copy(out=dst2[:, k, :], in_=dst[:, k, :])


def phase_a(cx, gi):
    """Inputs + sender attention + receiver k/v/q. Depends only on DMAs."""
    nc, w = cx.nc, cx.w
    px, pa, psm, pmi, ps = cx.px, cx.pa, cx.psm, cx.pmi, cx.ps
    st = Cx()
    st.gi = gi

    st.xT = px.tile([128, DC, T], BF, name="xT")
    nc.sync.dma_start(
        out=st.xT, in_=cx.zt[gi * 128:(gi + 1) * 128, :]
        .rearrange("p (c t) -> p c t", c=DC))
    st.xTf = px.tile([128, DC, T], FP, name="xTf")
    nc.sync.dma_start(
        out=st.xTf, in_=cx.ztf[gi * 128:(gi + 1) * 128, :]
        .rearrange("p (c t) -> p c t", c=DC))
    x_tok = px.tile([L, G, D], BF, name="x_tok")
    nc.sync.dma_start(
        out=x_tok, in_=cx.ztok[gi * L:(gi + 1) * L, :]
        .rearrange("p (g d) -> p g d", g=G))

    # ---- sender scores + softmax over tokens ----
    sc_ps = ps.tile([HR, T], FP, name="sc_ps", tag="sp", bufs=2)
    for k in range(DC):
        nc.tensor.matmul(out=sc_ps, lhsT=w["Msc"][:, k, :],
                         rhs=st.xT[:, k, :],
                         start=(k == 0), stop=(k == DC - 1))
    e1 = psm.tile([HR, T], BF, name="e1")
    nc.scalar.activation(out=e1, in_=sc_ps, func=ACTF.Exp, bias=w["c_score"])
    s1sum = pmi.tile([HR, G], FP, name="s1sum", tag="micro2")
    nc.vector.tensor_reduce(out=s1sum,
                            in_=e1.rearrange("p (g l) -> p g l", g=G),
                            axis=AX.X, op=OP.add)
    r1 = pmi.tile([HR, G], FP, name="r1", tag="micro2")
    nc.vector.reciprocal(out=r1, in_=s1sum)

    a1s = []
    for g in range(G):
        a1n = psm.tile([HR, L], BF, name=f"a1n{g}", tag="a1n", bufs=4)
        nc.vector.tensor_scalar_mul(out=a1n, in0=e1[:, g * L:(g + 1) * L],
                                    scalar1=r1[:, g:g + 1])
        a1p = ps.tile([L, HR], BF, name="a1p", tag="sp", bufs=2)
        nc.tensor.transpose(out=a1p, in_=a1n, identity=cx.identb[:HR, :HR])
        a1g = psm.tile([L, HR], BF, name=f"a1s{g}", tag="a1s", bufs=4)
        nc.scalar.copy(out=a1g, in_=a1p)
        a1s.append(a1g)

    # ---- T_mix^T [(dc),(g,hr)] = x^T @ A1^T (contract tokens, per g) ----
    tm_ps = ps.tile([128, DC, G, HR], FP, name="tm_ps", tag="sp", bufs=2)
    for g in range(G):
        for dc in range(DC):
            nc.tensor.matmul(out=tm_ps[:, dc, g, :],
                             lhsT=x_tok[:, g, dc * 128:(dc + 1) * 128],
                             rhs=a1s[g], start=True, stop=True)
    TmT = psm.tile([128, DC, G, HR], BF, name="TmT")
    nc.scalar.copy(out=TmT, in_=tm_ps)

    # ---- out_cat chunk h = Wv_s_h^T @ Tm_h ----
    oc_ps = ps.tile([128, H, G, R], FP, name="oc_ps", tag="sp", bufs=2)
    for h in range(H):
        for k in range(DC):
            nc.tensor.matmul(out=oc_ps[:, h, :, :],
                             lhsT=w["Wv_s"][:, k, h * E:(h + 1) * E],
                             rhs=TmT[:, k, :, h * R:(h + 1) * R],
                             start=(k == 0), stop=(k == DC - 1))
    Oc = psm.tile([128, H, G, R], BF, name="Oc")
    nc.scalar.copy(out=Oc, in_=oc_ps)

    # ---- router_buffer^T = Wo_s^T @ out_cat + c_send ----
    rb_ps = ps.tile([128, DC, G, R], FP, name="rb_ps", tag="sp", bufs=2)
    for dc in range(DC):
        for h in range(H):
            nc.tensor.matmul(out=rb_ps[:, dc, :, :],
                             lhsT=w["Wo_s"][:, h, dc * 128:(dc + 1) * 128],
                             rhs=Oc[:, h, :, :],
                             start=(h == 0), stop=(h == H - 1))
    rb = psm.tile([128, DC, G, R], BF, name="rb")
    for dc in range(DC):
        nc.vector.tensor_scalar_add(out=rb[:, dc, :, :],
                                    in0=rb_ps[:, dc, :, :],
                                    scalar1=w["c_send"][:, dc:dc + 1])

    # ---- receiver k^T [(h),(g,r)] ----
    kt_ps = ps.tile([128, H, G, R], FP, name="kt_ps", tag="sp", bufs=2)
    for h in range(H):
        for k in range(DC):
            nc.tensor.matmul(out=kt_ps[:, h, :, :],
                             lhsT=w["Wk_r"][:, k, h * E:(h + 1) * E],
                             rhs=rb[:, k, :, :],
                             start=(k == 0), stop=(k == DC - 1))
    st.kT = psm.tile([128, H, G, R], BF, name="kT")
    for h in range(H):
        nc.vector.tensor_scalar_add(out=st.kT[:, h, :, :],
                                    in0=kt_ps[:, h, :, :],
                                    scalar1=w["bk_r"][:, h:h + 1])

    # ---- receiver v, all 4 sequences: [(g,r), D] ----
    v_ps = ps.tile([GR, D], FP, name="v_ps", tag="sp", bufs=2)
    for k in range(DC):
        nc.tensor.matmul(out=v_ps, lhsT=rb[:, k, :, :],
                         rhs=w["Wv_r"][:, k, :],
                         start=(k == 0), stop=(k == DC - 1))
    st.v32 = psm.tile([GR, D], BF, name="v32")
    nc.scalar.copy(out=st.v32, in_=v_ps)

    # ---- receiver q^T [(h), T] (scale+bias pre-folded) ----
    st.qT = pa.tile([128, DC, T], BF, name="qT")
    for dc in range(DC):
        q_ps = ps.tile([128, T], FP, name="q_ps", tag="big", bufs=2)
        for k in range(DC):
            nc.tensor.matmul(out=q_ps,
                             lhsT=w["Wq_r"][:, k, dc * 128:(dc + 1) * 128],
                             rhs=st.xT[:, k, :], start=(k == 0),
                             stop=(k == DC - 1))
        nc.vector.tensor_scalar_add(out=st.qT[:, dc, :], in0=q_ps,
                                    scalar1=w["bq_r"][:, dc:dc + 1])
    return st


def phase_b_scores(cx, st):
    """Receiver scores -> masked exp -> denominators (per head)."""
    nc = cx.nc
    psm, pmi, ps = cx.psm, cx.pmi, cx.ps
    st.e2ts = []
    st.r2a = pmi.tile([1, H, T], FP, name="r2a", tag="r2a", bufs=2)
    for h in range(H):
        s2t_ps = ps.tile([HR, T], FP, name="s2t_ps", tag="sp", bufs=2)
        nc.tensor.matmul(out=s2t_ps, lhsT=st.kT[:, h, :, :],
                         rhs=st.qT[:, h, :], start=True, stop=True)
        et = psm.tile([HR, T], BF, name="et", tag="et", bufs=2)
        nc.scalar.activation(out=et, in_=s2t_ps, func=ACTF.Exp)
        e2t = psm.tile([HR, T], BF, name=f"e2t{h}", tag="e2t", bufs=6)
        nc.vector.tensor_mul(out=e2t, in0=et, in1=cx.mask32)
        st.e2ts.append(e2t)
    for h in range(H):
        den_ps = ps.tile([1, T], FP, name="den_ps", tag="sp", bufs=2)
        nc.tensor.matmul(out=den_ps, lhsT=cx.ones32, rhs=st.e2ts[h],
                         start=True, stop=True)
        nc.vector.reciprocal_approx_fast(out=st.r2a[:, h, :], in_=den_ps)


def phase_b_mix(cx, st):
    """Broadcast softmax scales; value mix per head."""
    nc, pb, pa, ps = cx.nc, cx.pb, cx.pa, cx.ps
    r2B = pb.tile([128, H, T], FP, name="r2B", tag="r2B", bufs=1)
    nc.gpsimd.partition_broadcast(r2B, st.r2a)
    st.aT = pa.tile([128, H, T], BF, name="aT")
    for h in range(H):
        mix_ps = ps.tile([128, T], FP, name="mix_ps", tag="big", bufs=2)
        nc.tensor.matmul(out=mix_ps, lhsT=st.v32[:, h * E:(h + 1) * E],
                         rhs=st.e2ts[h], start=True, stop=True)
        nc.vector.tensor_mul(out=st.aT[:, h, :], in0=mix_ps, in1=r2B[:, h, :])


def phase_b_tail(cx, st):
    """Output projection + residual + LN1 -> n1f (fp32) / n1T (bf16)."""
    nc, w, pa, ps = cx.nc, cx.w, cx.pa, cx.ps
    st.s1T = pa.tile([128, DC, T], FPR, name="s1T")
    for dc in range(DC):
        at2_ps = ps.tile([128, T], FP, name="at2_ps", tag="big", bufs=2)
        for k in range(DC):
            nc.tensor.matmul(out=at2_ps,
                             lhsT=w["Wo_r"][:, k, dc * 128:(dc + 1) * 128],
                             rhs=st.aT[:, k, :], start=(k == 0),
                             stop=(k == DC - 1))
        nc.vector.scalar_tensor_tensor(out=st.s1T[:, dc, :], in0=at2_ps,
                                       scalar=w["c_recv"][:, dc:dc + 1],
                                       in1=st.xTf[:, dc, :],
                                       op0=OP.add, op1=OP.add)
    st.n1f = pa.tile([128, DC, T], FP, name="n1f")
    st.n1T = pa.tile([128, DC, T], BF, name="n1T")
    ln_norm(cx, st.s1T, st.n1f, "_l1", dst2=st.n1T)


def phase_c_mlp(cx, st, oc0, oc1):
    """MLP hidden chunks [oc0, oc1)."""
    nc, w, pb, ps = cx.nc, cx.w, cx.pb, cx.ps
    if oc0 == 0:
        st.h2_ps = [ps.tile([128, T], FP, name=f"h2_ps{dc}", tag=f"h2_{dc}",
                            bufs=1) for dc in range(DC)]
    for oc in range(oc0, oc1):
        h1_ps = ps.tile([128, T], FP, name="h1_ps", tag="big", bufs=2)
        for k in range(DC):
            nc.tensor.matmul(out=h1_ps,
                             lhsT=w["W1"][:, k, oc * 128:(oc + 1) * 128],
                             rhs=st.n1T[:, k, :], start=(k == 0),
                             stop=(k == DC - 1))
        gl = pb.tile([128, T], BF, name="gl", tag="gl", bufs=3)
        nc.scalar.activation(out=gl, in_=h1_ps, func=ACTF.Gelu,
                             bias=w["b1"][:, oc:oc + 1])
        for dc in range(DC):
            nc.tensor.matmul(out=st.h2_ps[dc],
                             lhsT=w["W2"][:, oc, dc * 128:(dc + 1) * 128],
                             rhs=gl, start=(oc == 0), stop=(oc == OC - 1))


def phase_c_tail(cx, st):
    """Rest of MLP + residual2 + LN2 -> fp16 transposed output + DMA."""
    nc, w, pa, pb, po = cx.nc, cx.w, cx.pa, cx.pb, cx.po
    phase_c_mlp(cx, st, OC // 2, OC)
    s2T = pa.tile([128, DC, T], FPR, name="s2T")
    for dc in range(DC):
        u = pb.tile([128, T], FP, name="u", tag="u", bufs=2)
        nc.vector.tensor_scalar(out=u, in0=st.n1f[:, dc, :],
                                scalar1=w["ln1_g"][:, dc:dc + 1], op0=OP.mult,
                                scalar2=w["b2p"][:, dc:dc + 1], op1=OP.add)
        nc.vector.tensor_add(out=s2T[:, dc, :], in0=st.h2_ps[dc], in1=u)
    outT = po.tile([128, DC, T], F16, name="outT")
    ln_norm(cx, s2T, outT, "_l2")
    nc.gpsimd.dma_start(
        out=cx.out[st.gi * 128:(st.gi + 1) * 128, :]
        .rearrange("p (c t) -> p c t", c=DC),
        in_=outT)


# ---------------------------------------------------------------- host side

def _host_fold(inputs):
    f32 = np.float32
    scale = 1.0 / np.sqrt(np.float32(E))
    q_s = (inputs["router"] @ inputs["Wq_s"] + inputs["bq_s"]).astype(f32)
    q_sh = q_s.reshape(R, H, E)
    Wk = inputs["Wk_s"].reshape(D, H, E)
    M_score = (np.einsum("dhe,rhe->dhr", Wk, q_sh).reshape(D, HR)
               * scale).astype(f32)
    c_score = (np.einsum("he,rhe->hr", inputs["bk_s"].reshape(H, E), q_sh)
               .reshape(HR) * scale).astype(f32)
    c_send = (inputs["bv_s"] @ inputs["Wo_s"] + inputs["bo_s"]).astype(f32)
    c_recv = (inputs["bv_r"] @ inputs["Wo_r"] + inputs["bo_r"]).astype(f32)
    W1f = (inputs["ln1_g"][:, None] * inputs["W1"]).astype(f32)
    b1f = (inputs["b1"] + inputs["ln1_b"] @ inputs["W1"]).astype(f32)
    b2p = (inputs["b2"] + inputs["ln1_b"]).astype(f32)

    def bf(x):
        return np.ascontiguousarray(np.asarray(x, f32).astype(BF_NP))

    return {
        "Msc": bf(M_score),
        "Wv_s": bf(inputs["Wv_s"]),
        "Wo_s": bf(inputs["Wo_s"]),
        "Wq_r": bf(inputs["Wq_r"] * scale),
        "Wk_r": bf(inputs["Wk_r"]),
        "Wv_r": bf(inputs["Wv_r"]),
        "Wo_r": bf(inputs["Wo_r"]),
        "W1": bf(W1f),
        "W2": bf(inputs["W2"]),
        "c_score": c_score,
        "c_send": c_send,
        "c_recv": c_recv,
        "bq_r": (inputs["bq_r"] * scale).astype(f32),
        "bk_r": inputs["bk_r"].astype(f32),
        "b1": b1f,
        "ln1_g": inputs["ln1_g"].astype(f32),
        "b2p": b2p,
        "mask": _block_mask(),
    }


def _block_mask():
    m = np.zeros((HR, T), np.float32)
    for g in range(G):
        m[g * R:(g + 1) * R, g * L:(g + 1) * L] = 1.0
    return m.astype(BF_NP)


def _prep_core_inputs(Zc, folded):
    """Zc: [n_seq, L, D] fp32 -> {'zt','ztf','ztok'} for one core."""
    n_seq = Zc.shape[0]
    ng = n_seq // G
    Zg = Zc.reshape(ng, G, L, DC, 128)
    ztf = np.ascontiguousarray(
        Zg.transpose(0, 4, 3, 1, 2).reshape(ng * 128, DC * T))
    zt = ztf.astype(BF_NP)
    ztok = np.ascontiguousarray(
        Zc.reshape(ng, G, L, D).transpose(0, 2, 1, 3)
        .reshape(ng * L, G * D)).astype(BF_NP)
    m = {"zt": zt, "ztf": ztf, "ztok": ztok}
    m.update(folded)
    return m


def _post_core_output(o, ln2_g, ln2_b):
    """o: [ng*128, DC*T] fp16 -> [n_seq, L, D] fp32 (LN2 affine applied)."""
    ng = o.shape[0] // 128
    x = o.astype(np.float32).reshape(ng, 128, DC, G, L)
    x = x.transpose(0, 3, 4, 2, 1).reshape(ng * G, L, D)
    return x * ln2_g[None, None, :] + ln2_b[None, None, :]


def kernel(**inputs) -> np.ndarray:
    inputs = {k: np.asarray(v) for k, v in inputs.items()}
    Z = inputs["Z"].astype(np.float32)
    n_seq_total = B * C
    n_seq = n_seq_total // N_CORES
    folded = _host_fold(inputs)

    nc = build_core_kernel(n_seq)
    Zf = Z.reshape(n_seq_total, L, D)
    in_maps = [_prep_core_inputs(Zf[c * n_seq:(c + 1) * n_seq], folded)
               for c in range(N_CORES)]
    res = run_bass_kernel_spmd(nc, in_maps, list(range(N_CORES)))
    ln2_g = inputs["ln2_g"].astype(np.float32)
    ln2_b = inputs["ln2_b"].astype(np.float32)
    out = np.empty((n_seq_total, L, D), np.float32)
    for c in range(N_CORES):
        out[c * n_seq:(c + 1) * n_seq] = _post_core_output(
            res.results[c]["out"], ln2_g, ln2_b)
    return out.reshape(B, C, L, D)


if __name__ == "__main__":
    import reference
    inputs = {k: np.asarray(v) for k, v in reference.setup_inputs().items()}
    expected = np.asarray(reference.reference(**inputs))
    got = kernel(**inputs)
    err = np.abs(got - expected).max()
    rel = err / np.abs(expected).max()
    print(f"abs err {err:.3e}  absmax-rel {rel:.3e}")


# revision 61
# speedup vs baseline: 2.0877x; 1.0624x over previous
"""CrossPhaseRoutingLayer Trainium2 kernel (v3: bf16 + software pipelining).

Full inputs -> full output. Data-parallel over the fused B*C=512 sequence axis
across 8 NeuronCores (64 sequences each); per core, 16 groups of G=4 sequences
(T = 384 token columns per group).

Design highlights:
  - all matmul operands bf16 (4x faster small matmuls, FWL weight loads);
    residual/LN tensors fp32 (fp32r for matmul reads)
  - host-side pre-transpose of Z in both layouts (no on-chip input/output
    transposes); output stored transposed fp16, untransposed on host
  - sender scores collapse into one matrix (router q is input-independent);
    sender value/output path runs mix-first; receiver attention batched over
    the 4 sequences with a block-diagonal exp mask (12 matmuls/group)
  - softmax/LN reciprocals via DVE reciprocal_approx_fast (no ACT table
    thrash; DVE reciprocal on [1,T] rows is ~3.2us each)
  - LN1 gamma/beta folded into W1/b1/b2; LN2 affine applied on host
  - group phases software-pipelined at emission time so group i's MLP
    matmuls hide group i+1's softmax/LN cross-engine latency (the PE
    executes its stream in order - without interleaving every ACT/DVE
    round-trip stalls it)
"""
import numpy as np

import concourse.bacc as bacc
import concourse.bass as bass
import concourse.mybir as mybir
import concourse.tile as tile
from concourse.bass_utils import run_bass_kernel_spmd
from concourse.masks import make_identity

FP = mybir.dt.float32
FPR = mybir.dt.float32r
BF = mybir.dt.bfloat16
F16 = mybir.dt.float16
AX = mybir.AxisListType
OP = mybir.AluOpType
ACTF = mybir.ActivationFunctionType

B, C, L, D = 16, 32, 96, 512
R, H = 8, 4
E = D // H            # 128
HR = H * R            # 32
GR = 32               # G * R
DC = D // 128         # 4 chunks of D
OC = (4 * D) // 128   # 16 MLP hidden chunks
EPS = 1e-5
N_CORES = 8
G = 4                 # sequences per group
T = G * L             # 384 token columns per group

BF_NP = mybir.dt.np(BF)
F16_NP = mybir.dt.np(F16)

W_BF = [("Msc", [D, HR]), ("Wv_s", [D, D]), ("Wo_s", [D, D]),
        ("Wq_r", [D, D]), ("Wk_r", [D, D]), ("Wv_r", [D, D]),
        ("Wo_r", [D, D]), ("W1", [D, 4 * D]), ("W2", [4 * D, D])]
V_FP = [("c_score", HR), ("c_send", D), ("c_recv", D), ("bq_r", D),
        ("bk_r", D), ("b1", 4 * D), ("ln1_g", D), ("b2p", D)]


class Cx:
    """Shared emission context (engines, weights, pools, dram handles)."""
    pass


def build_core_kernel(n_seq: int):
    assert n_seq % G == 0
    n_groups = n_seq // G
    nc = bacc.Bacc(None)

    cx = Cx()
    cx.nc = nc
    cx.zt = nc.declare_dram_parameter("zt", [n_groups * 128, DC * T], BF,
                                      isOutput=False)
    cx.ztf = nc.declare_dram_parameter("ztf", [n_groups * 128, DC * T], FP,
                                       isOutput=False)
    cx.ztok = nc.declare_dram_parameter("ztok", [n_groups * L, G * D], BF,
                                        isOutput=False)
    cx.out = nc.declare_dram_parameter("out", [n_groups * 128, DC * T], F16,
                                       isOutput=True)
    wd = {}
    for name, shape in W_BF:
        wd[name] = nc.declare_dram_parameter(name, shape, BF, isOutput=False)
    for name, n in V_FP:
        wd[name] = nc.declare_dram_parameter(name, [n], FP, isOutput=False)
    wd["mask"] = nc.declare_dram_parameter("mask", [HR, T], BF, isOutput=False)

    with tile.TileContext(nc) as tc:
        with tc.tile_pool(name="wpool", bufs=1) as wp, \
             tc.tile_pool(name="xin", bufs=2) as px, \
             tc.tile_pool(name="act", bufs=2) as pa, \
             tc.tile_pool(name="sm", bufs=2) as psm, \
             tc.tile_pool(name="micro", bufs=2) as pmi, \
             tc.tile_pool(name="brd", bufs=2) as pb, \
             tc.tile_pool(name="otok", bufs=2) as po, \
             tc.tile_pool(name="ps", bufs=1, space="PSUM") as ps:

            cx.px, cx.pa, cx.psm, cx.pmi, cx.pb, cx.po, cx.ps = \
                px, pa, psm, pmi, pb, po, ps

            # ------------- resident weights / constants (scalar queue) ------
            w = {}
            w["Msc"] = wp.tile([128, DC, HR], BF, name="w_Msc")
            for name in ["Wv_s", "Wo_s", "Wq_r", "Wk_r", "Wv_r", "Wo_r"]:
                w[name] = wp.tile([128, DC, D], BF, name=f"w_{name}")
            w["W1"] = wp.tile([128, DC, 4 * D], BF, name="w_W1")
            w["W2"] = wp.tile([128, OC, D], BF, name="w_W2")
            for name, _ in W_BF:
                nc.scalar.dma_start(
                    out=w[name],
                    in_=wd[name].rearrange("(c p) x -> p c x", p=128))
            w["c_score"] = wp.tile([HR, 1], FP, name="w_c_score")
            nc.scalar.dma_start(out=w["c_score"],
                                in_=wd["c_score"].rearrange("(p o) -> p o", o=1))
            for name in ["c_send", "c_recv", "bq_r", "bk_r", "ln1_g", "b2p"]:
                w[name] = wp.tile([128, DC], FP, name=f"w_{name}")
                nc.scalar.dma_start(out=w[name],
                                    in_=wd[name].rearrange("(c p) -> p c", p=128))
            w["b1"] = wp.tile([128, OC], FP, name="w_b1")
            nc.scalar.dma_start(out=w["b1"],
                                in_=wd["b1"].rearrange("(c p) -> p c", p=128))
            cx.w = w

            ident = wp.tile([128, 128], FP, name="ident")
            make_identity(nc, ident)
            cx.identb = wp.tile([128, 128], BF, name="identb")
            nc.scalar.copy(out=cx.identb, in_=ident)
            cx.ones32 = wp.tile([HR, 1], BF, name="ones32")
            nc.vector.memset(cx.ones32, 1.0)
            cx.invD = wp.tile([128, 1], BF, name="invD")
            nc.vector.memset(cx.invD, 1.0 / D)
            invDf = wp.tile([128, 1], FP, name="invDf")
            nc.vector.memset(invDf, 1.0 / D)
            cx.invDr = invDf.bitcast(FPR)
            cx.mask32 = wp.tile([HR, T], BF, name="mask32")
            nc.scalar.dma_start(out=cx.mask32, in_=wd["mask"][:, :])

            # ---------------- software-pipelined group loop -----------------
            sts = {0: phase_a_dma(cx, 0)}
            phase_a_scores(cx, sts[0])
            phase_a_heavy(cx, sts[0])
            for gi in range(n_groups):
                st = sts[gi]
                if gi + 1 < n_groups:
                    sts[gi + 1] = phase_a_dma(cx, gi + 1)
                if gi > 0:
                    phase_c_mlp(cx, sts[gi - 1], 0, 6)
                phase_b_scores(cx, st)
                if gi > 0:
                    phase_c_mlp(cx, sts[gi - 1], 6, 11)
                phase_b_mix(cx, st)
                if gi > 0:
                    phase_c_tail(cx, sts[gi - 1])
                if gi + 1 < n_groups:
                    phase_a_scores(cx, sts[gi + 1])
                phase_b_tail(cx, st)
                if gi + 1 < n_groups:
                    phase_a_heavy(cx, sts[gi + 1])
                if gi - 1 in sts:
                    del sts[gi - 1]
            phase_c_mlp(cx, sts[n_groups - 1], 0, 6)
            phase_c_mlp(cx, sts[n_groups - 1], 6, 11)
            phase_c_tail(cx, sts[n_groups - 1])
    nc.finalize()
    return nc


def ln_stats_start(cx, tag):
    ps = cx.ps
    m_ps = ps.tile([1, T], FP, name=f"m_ps{tag}", tag="sp", bufs=2)
    s_ps = ps.tile([1, T], FP, name=f"s_ps{tag}", tag="sp", bufs=2)
    return m_ps, s_ps


def ln_stats_chunk(cx, stats, src, k, tag):
    """Accumulate mean / mean-square contributions of chunk k."""
    nc, pb = cx.nc, cx.pb
    m_ps, s_ps = stats
    nc.tensor.matmul(out=m_ps, lhsT=cx.invDr, rhs=src[:, k, :],
                     start=(k == 0), stop=(k == DC - 1))
    sq = pb.tile([128, T], BF, name=f"sq{tag}", tag="sq", bufs=2)
    nc.vector.tensor_mul(out=sq, in0=src[:, k, :].bitcast(FP),
                         in1=src[:, k, :].bitcast(FP))
    nc.tensor.matmul(out=s_ps, lhsT=cx.invD, rhs=sq,
                     start=(k == 0), stop=(k == DC - 1))


def ln_finish(cx, stats, src, dst, tag, dst2=None):
    """Row math + broadcast + normalize: dst = (src - m)/sqrt(var+eps)."""
    nc, pmi, pb = cx.nc, cx.pmi, cx.pb
    m_ps, s_ps = stats
    m2 = pmi.tile([1, T], FP, name=f"m2{tag}", tag="microln")
    nc.scalar.square(out=m2, in_=m_ps)
    veps = pmi.tile([1, T], FP, name=f"veps{tag}", tag="microln")
    nc.vector.scalar_tensor_tensor(out=veps, in0=s_ps, scalar=EPS,
                                   in1=m2, op0=OP.add, op1=OP.subtract)
    srow = pmi.tile([1, T], FP, name=f"srow{tag}", tag="microln")
    nc.scalar.sqrt(out=srow, in_=veps)
    lrow = pmi.tile([1, 2, T], FP, name=f"lrow{tag}", tag="lrow")
    nc.vector.reciprocal_approx_fast(out=lrow[:, 1, :], in_=srow)
    nc.vector.tensor_mul(out=lrow[:, 0, :], in0=m_ps, in1=lrow[:, 1, :])
    lnB = pb.tile([128, 2, T], FP, name=f"lnB{tag}", tag="lnB", bufs=2)
    nc.gpsimd.partition_broadcast(lnB, lrow)
    for k in range(DC):
        t = pb.tile([128, T], FP, name=f"lt{tag}", tag="lnt", bufs=2)
        nc.vector.tensor_mul(out=t, in0=src[:, k, :].bitcast(FP),
                             in1=lnB[:, 1, :])
        nc.vector.tensor_sub(out=dst[:, k, :], in0=t, in1=lnB[:, 0, :])
        if dst2 is not None:
            nc.scalar.copy(out=dst2[:, k, :], in_=dst[:, k, :])


def phase_a_dma(cx, gi):
    """Post the group's input DMAs (early, for prefetch)."""
    nc, px = cx.nc, cx.px
    st = Cx()
    st.gi = gi
    st.xT = px.tile([128, DC, T], BF, name="xT")
    nc.sync.dma_start(
        out=st.xT, in_=cx.zt[gi * 128:(gi + 1) * 128, :]
        .rearrange("p (c t) -> p c t", c=DC))
    st.xTf = px.tile([128, DC, T], FP, name="xTf")
    nc.sync.dma_start(
        out=st.xTf, in_=cx.ztf[gi * 128:(gi + 1) * 128, :]
        .rearrange("p (c t) -> p c t", c=DC))
    st.x_tok = px.tile([L, G, D], BF, name="x_tok")
    nc.sync.dma_start(
        out=st.x_tok, in_=cx.ztok[gi * L:(gi + 1) * L, :]
        .rearrange("p (g d) -> p g d", g=G))
    return st


def phase_a_scores(cx, st):
    """Sender scores + softmax weights (cheap PE, ACT/DVE latency)."""
    nc, w = cx.nc, cx.w
    psm, pmi, ps = cx.psm, cx.pmi, cx.ps
    sc_ps = ps.tile([HR, T], FP, name="sc_ps", tag="sp", bufs=2)
    for k in range(DC):
        nc.tensor.matmul(out=sc_ps, lhsT=w["Msc"][:, k, :],
                         rhs=st.xT[:, k, :],
                         start=(k == 0), stop=(k == DC - 1))
    e1 = psm.tile([HR, T], BF, name="e1")
    nc.scalar.activation(out=e1, in_=sc_ps, func=ACTF.Exp, bias=w["c_score"])
    s1sum = pmi.tile([HR, G], FP, name="s1sum", tag="micro2")
    nc.vector.tensor_reduce(out=s1sum,
                            in_=e1.rearrange("p (g l) -> p g l", g=G),
                            axis=AX.X, op=OP.add)
    r1 = pmi.tile([HR, G], FP, name="r1", tag="micro2")
    nc.vector.reciprocal(out=r1, in_=s1sum)
    st.a1n = []
    for g in range(G):
        a1n = psm.tile([HR, L], BF, name=f"a1n{g}", tag="a1n", bufs=8)
        nc.vector.tensor_scalar_mul(out=a1n, in0=e1[:, g * L:(g + 1) * L],
                                    scalar1=r1[:, g:g + 1])
        st.a1n.append(a1n)


def phase_a_heavy(cx, st):
    """Sender mix/projections + receiver k/v/q (PE-heavy)."""
    nc, w = cx.nc, cx.w
    pa, psm, ps = cx.pa, cx.psm, cx.ps
    x_tok = st.x_tok

    a1s = []
    for g in range(G):
        a1p = ps.tile([L, HR], BF, name="a1p", tag="sp", bufs=2)
        nc.tensor.transpose(out=a1p, in_=st.a1n[g],
                            identity=cx.identb[:HR, :HR])
        a1g = psm.tile([L, HR], BF, name=f"a1s{g}", tag="a1s", bufs=4)
        nc.scalar.copy(out=a1g, in_=a1p)
        a1s.append(a1g)

    # ---- T_mix^T [(dc),(g,hr)] = x^T @ A1^T (contract tokens, per g) ----
    tm_ps = ps.tile([128, DC, G, HR], FP, name="tm_ps", tag="sp", bufs=2)
    for g in range(G):
        for dc in range(DC):
            nc.tensor.matmul(out=tm_ps[:, dc, g, :],
                             lhsT=x_tok[:, g, dc * 128:(dc + 1) * 128],
                             rhs=a1s[g], start=True, stop=True)
    TmT = psm.tile([128, DC, G, HR], BF, name="TmT")
    nc.scalar.copy(out=TmT, in_=tm_ps)

    # ---- out_cat chunk h = Wv_s_h^T @ Tm_h ----
    oc_ps = ps.tile([128, H, G, R], FP, name="oc_ps", tag="sp", bufs=2)
    for h in range(H):
        for k in range(DC):
            nc.tensor.matmul(out=oc_ps[:, h, :, :],
                             lhsT=w["Wv_s"][:, k, h * E:(h + 1) * E],
                             rhs=TmT[:, k, :, h * R:(h + 1) * R],
                             start=(k == 0), stop=(k == DC - 1))
    Oc = psm.tile([128, H, G, R], BF, name="Oc")
    nc.scalar.copy(out=Oc, in_=oc_ps)

    # ---- router_buffer^T = Wo_s^T @ out_cat + c_send ----
    rb_ps = ps.tile([128, DC, G, R], FP, name="rb_ps", tag="sp", bufs=2)
    for dc in range(DC):
        for h in range(H):
            nc.tensor.matmul(out=rb_ps[:, dc, :, :],
                             lhsT=w["Wo_s"][:, h, dc * 128:(dc + 1) * 128],
                             rhs=Oc[:, h, :, :],
                             start=(h == 0), stop=(h == H - 1))
    rb = psm.tile([128, DC, G, R], BF, name="rb")
    for dc in range(DC):
        nc.vector.tensor_scalar_add(out=rb[:, dc, :, :],
                                    in0=rb_ps[:, dc, :, :],
                                    scalar1=w["c_send"][:, dc:dc + 1])

    # ---- receiver k^T [(h),(g,r)] ----
    kt_ps = ps.tile([128, H, G, R], FP, name="kt_ps", tag="sp", bufs=2)
    for h in range(H):
        for k in range(DC):
            nc.tensor.matmul(out=kt_ps[:, h, :, :],
                             lhsT=w["Wk_r"][:, k, h * E:(h + 1) * E],
                             rhs=rb[:, k, :, :],
                             start=(k == 0), stop=(k == DC - 1))
    st.kT = psm.tile([128, H, G, R], BF, name="kT")
    for h in range(H):
        nc.vector.tensor_scalar_add(out=st.kT[:, h, :, :],
                                    in0=kt_ps[:, h, :, :],
                                    scalar1=w["bk_r"][:, h:h + 1])

    # ---- receiver v, all 4 sequences: [(g,r), D] ----
    v_ps = ps.tile([GR, D], FP, name="v_ps", tag="sp", bufs=2)
    for k in range(DC):
        nc.tensor.matmul(out=v_ps, lhsT=rb[:, k, :, :],
                         rhs=w["Wv_r"][:, k, :],
                         start=(k == 0), stop=(k == DC - 1))
    st.v32 = psm.tile([GR, D], BF, name="v32")
    nc.scalar.copy(out=st.v32, in_=v_ps)

    # ---- receiver q^T [(h), T] (scale+bias pre-folded) ----
    st.qT = pa.tile([128, DC, T], BF, name="qT")
    for dc in range(DC):
        q_ps = ps.tile([128, T], FP, name="q_ps", tag="big", bufs=2)
        for k in range(DC):
            nc.tensor.matmul(out=q_ps,
                             lhsT=w["Wq_r"][:, k, dc * 128:(dc + 1) * 128],
                             rhs=st.xT[:, k, :], start=(k == 0),
                             stop=(k == DC - 1))
        nc.vector.tensor_scalar_add(out=st.qT[:, dc, :], in0=q_ps,
                                    scalar1=w["bq_r"][:, dc:dc + 1])
    return st


def phase_b_scores(cx, st):
    """Receiver scores -> masked exp -> denominators (per head)."""
    nc = cx.nc
    psm, pmi, ps = cx.psm, cx.pmi, cx.ps
    st.e2ts = []
    st.r2a = pmi.tile([1, H, T], FP, name="r2a", tag="r2a", bufs=2)
    for h in range(H):
        s2t_ps = ps.tile([HR, T], FP, name="s2t_ps", tag="sp", bufs=2)
        nc.tensor.matmul(out=s2t_ps, lhsT=st.kT[:, h, :, :],
                         rhs=st.qT[:, h, :], start=True, stop=True)
        et = psm.tile([HR, T], BF, name="et", tag="et", bufs=2)
        nc.scalar.activation(out=et, in_=s2t_ps, func=ACTF.Exp)
        e2t = psm.tile([HR, T], BF, name=f"e2t{h}", tag="e2t", bufs=6)
        nc.vector.tensor_mul(out=e2t, in0=et, in1=cx.mask32)
        st.e2ts.append(e2t)
    for h in range(H):
        den_ps = ps.tile([1, T], FP, name="den_ps", tag="sp", bufs=2)
        nc.tensor.matmul(out=den_ps, lhsT=cx.ones32, rhs=st.e2ts[h],
                         start=True, stop=True)
        nc.vector.reciprocal_approx_fast(out=st.r2a[:, h, :], in_=den_ps)


def phase_b_mix(cx, st):
    """Broadcast softmax scales; value mix per head."""
    nc, pb, pa, ps = cx.nc, cx.pb, cx.pa, cx.ps
    r2B = pb.tile([128, H, T], FP, name="r2B", tag="r2B", bufs=1)
    nc.gpsimd.partition_broadcast(r2B, st.r2a)
    st.aT = pa.tile([128, H, T], BF, name="aT")
    for h in range(H):
        mix_ps = ps.tile([128, T], FP, name="mix_ps", tag="big", bufs=2)
        nc.tensor.matmul(out=mix_ps, lhsT=st.v32[:, h * E:(h + 1) * E],
                         rhs=st.e2ts[h], start=True, stop=True)
        nc.vector.tensor_mul(out=st.aT[:, h, :], in0=mix_ps, in1=r2B[:, h, :])


def phase_b_tail(cx, st):
    """Output projection + residual + LN1 -> n1f (fp32) / n1T (bf16)."""
    nc, w, pa, ps = cx.nc, cx.w, cx.pa, cx.ps
    st.s1T = pa.tile([128, DC, T], FPR, name="s1T")
    stats = ln_stats_start(cx, "_l1")
    for dc in range(DC):
        at2_ps = ps.tile([128, T], FP, name="at2_ps", tag="big", bufs=2)
        for k in range(DC):
            nc.tensor.matmul(out=at2_ps,
                             lhsT=w["Wo_r"][:, k, dc * 128:(dc + 1) * 128],
                             rhs=st.aT[:, k, :], start=(k == 0),
                             stop=(k == DC - 1))
        nc.vector.scalar_tensor_tensor(out=st.s1T[:, dc, :], in0=at2_ps,
                                       scalar=w["c_recv"][:, dc:dc + 1],
                                       in1=st.xTf[:, dc, :],
                                       op0=OP.add, op1=OP.add)
        ln_stats_chunk(cx, stats, st.s1T, dc, "_l1")
    st.n1f = pa.tile([128, DC, T], FP, name="n1f")
    st.n1T = pa.tile([128, DC, T], BF, name="n1T")
    ln_finish(cx, stats, st.s1T, st.n1f, "_l1", dst2=st.n1T)


def phase_c_mlp(cx, st, oc0, oc1):
    """MLP hidden chunks [oc0, oc1)."""
    nc, w, pb, ps = cx.nc, cx.w, cx.pb, cx.ps
    if oc0 == 0:
        st.h2_ps = [ps.tile([128, T], FP, name=f"h2_ps{dc}", tag=f"h2_{dc}",
                            bufs=1) for dc in range(DC)]
    for oc in range(oc0, oc1):
        h1_ps = ps.tile([128, T], FP, name="h1_ps", tag="big", bufs=2)
        for k in range(DC):
            nc.tensor.matmul(out=h1_ps,
                             lhsT=w["W1"][:, k, oc * 128:(oc + 1) * 128],
                             rhs=st.n1T[:, k, :], start=(k == 0),
                             stop=(k == DC - 1))
        gl = pb.tile([128, T], BF, name="gl", tag="gl", bufs=3)
        nc.scalar.activation(out=gl, in_=h1_ps, func=ACTF.Gelu,
                             bias=w["b1"][:, oc:oc + 1])
        for dc in range(DC):
            nc.tensor.matmul(out=st.h2_ps[dc],
                             lhsT=w["W2"][:, oc, dc * 128:(dc + 1) * 128],
                             rhs=gl, start=(oc == 0), stop=(oc == OC - 1))


def phase_c_tail(cx, st):
    """Rest of MLP + residual2 + LN2 -> fp16 transposed output + DMA."""
    nc, w, pa, pb, po = cx.nc, cx.w, cx.pa, cx.pb, cx.po
    phase_c_mlp(cx, st, 11, OC)
    s2T = pa.tile([128, DC, T], FPR, name="s2T")
    stats = ln_stats_start(cx, "_l2")
    for dc in range(DC):
        u = pb.tile([128, T], FP, name="u", tag="u", bufs=2)
        nc.vector.tensor_scalar(out=u, in0=st.n1f[:, dc, :],
                                scalar1=w["ln1_g"][:, dc:dc + 1], op0=OP.mult,
                                scalar2=w["b2p"][:, dc:dc + 1], op1=OP.add)
        nc.vector.tensor_add(out=s2T[:, dc, :], in0=st.h2_ps[dc], in1=u)
        ln_stats_chunk(cx, stats, s2T, dc, "_l2")
    outT = po.tile([128, DC, T], F16, name="outT")
    ln_finish(cx, stats, s2T, outT, "_l2")
    nc.gpsimd.dma_start(
        out=cx.out[st.gi * 128:(st.gi + 1) * 128, :]
        .rearrange("p (c t) -> p c t", c=DC),
        in_=outT)


# ---------------------------------------------------------------- host side

def _host_fold(inputs):
    f32 = np.float32
    scale = 1.0 / np.sqrt(np.float32(E))
    q_s = (inputs["router"] @ inputs["Wq_s"] + inputs["bq_s"]).astype(f32)
    q_sh = q_s.reshape(R, H, E)
    Wk = inputs["Wk_s"].reshape(D, H, E)
    M_score = (np.einsum("dhe,rhe->dhr", Wk, q_sh).reshape(D, HR)
               * scale).astype(f32)
    c_score = (np.einsum("he,rhe->hr", inputs["bk_s"].reshape(H, E), q_sh)
               .reshape(HR) * scale).astype(f32)
    c_send = (inputs["bv_s"] @ inputs["Wo_s"] + inputs["bo_s"]).astype(f32)
    c_recv = (inputs["bv_r"] @ inputs["Wo_r"] + inputs["bo_r"]).astype(f32)
    W1f = (inputs["ln1_g"][:, None] * inputs["W1"]).astype(f32)
    b1f = (inputs["b1"] + inputs["ln1_b"] @ inputs["W1"]).astype(f32)
    b2p = (inputs["b2"] + inputs["ln1_b"]).astype(f32)

    def bf(x):
        return np.ascontiguousarray(np.asarray(x, f32).astype(BF_NP))

    return {
        "Msc": bf(M_score),
        "Wv_s": bf(inputs["Wv_s"]),
        "Wo_s": bf(inputs["Wo_s"]),
        "Wq_r": bf(inputs["Wq_r"] * scale),
        "Wk_r": bf(inputs["Wk_r"]),
        "Wv_r": bf(inputs["Wv_r"]),
        "Wo_r": bf(inputs["Wo_r"]),
        "W1": bf(W1f),
        "W2": bf(inputs["W2"]),
        "c_score": c_score,
        "c_send": c_send,
        "c_recv": c_recv,
        "bq_r": (inputs["bq_r"] * scale).astype(f32),
        "bk_r": inputs["bk_r"].astype(f32),
        "b1": b1f,
        "ln1_g": inputs["ln1_g"].astype(f32),
        "b2p": b2p,
        "mask": _block_mask(),
    }


def _block_mask():
    m = np.zeros((HR, T), np.float32)
    for g in range(G):
        m[g * R:(g + 1) * R, g * L:(g + 1) * L] = 1.0
    return m.astype(BF_NP)


def _prep_core_inputs(Zc, folded):
    """Zc: [n_seq, L, D] fp32 -> {'zt','ztf','ztok'} for one core."""
    n_seq = Zc.shape[0]
    ng = n_seq // G
    Zg = Zc.reshape(ng, G, L, DC, 128)
    ztf = np.ascontiguousarray(
        Zg.transpose(0, 4, 3, 1, 2).reshape(ng * 128, DC * T))
    zt = ztf.astype(BF_NP)
    ztok = np.ascontiguousarray(
        Zc.reshape(ng, G, L, D).transpose(0, 2, 1, 3)
        .reshape(ng * L, G * D)).astype(BF_NP)
    m = {"zt": zt, "ztf": ztf, "ztok": ztok}
    m.update(folded)
    return m


def _post_core_output(o, ln2_g, ln2_b):
    """o: [ng*128, DC*T] fp16 -> [n_seq, L, D] fp32 (LN2 affine applied)."""
    ng = o.shape[0] // 128
    x = o.astype(np.float32).reshape(ng, 128, DC, G, L)
    x = x.transpose(0, 3, 4, 2, 1).reshape(ng * G, L, D)
    return x * ln2_g[None, None, :] + ln2_b[None, None, :]


def kernel(**inputs) -> np.ndarray:
    inputs = {k: np.asarray(v) for k, v in inputs.items()}
    Z = inputs["Z"].astype(np.float32)
    n_seq_total = B * C
    n_seq = n_seq_total // N_CORES
    folded = _host_fold(inputs)

    nc = build_core_kernel(n_seq)
    Zf = Z.reshape(n_seq_total, L, D)
    in_maps = [_prep_core_inputs(Zf[c * n_seq:(c + 1) * n_seq], folded)
               for c in range(N_CORES)]
    res = run_bass_kernel_spmd(nc, in_maps, list(range(N_CORES)))
    ln2_g = inputs["ln2_g"].astype(np.float32)
    ln2_b = inputs["ln2_b"].astype(np.float32)
    out = np.empty((n_seq_total, L, D), np.float32)
    for c in range(N_CORES):
        out[c * n_seq:(c + 1) * n_seq] = _post_core_output(
            res.results[c]["out"], ln2_g, ln2_b)
    return out.reshape(B, C, L, D)


if __name__ == "__main__":
    import reference
    inputs = {k: np.asarray(v) for k, v in reference.setup_inputs().items()}
    expected = np.asarray(reference.reference(**inputs))
    got = kernel(**inputs)
    err = np.abs(got - expected).max()
    rel = err / np.abs(expected).max()
    print(f"abs err {err:.3e}  absmax-rel {rel:.3e}")
